# revision 21
# baseline (speedup 1.0000x reference)
"""Distributed Trainium2 (Bass) kernel for nn_Attention_53764400611491.

The reference module has HEADS == C == 64, so head_dim d = C//HEADS = 1.
With d = 1 the attention algebra collapses: per (batch b, head c)

    attn = q k^T            (outer product, [N,N])
    o    = attn @ v  =  q * (k . v)        <- a scalar per (b,c)!

so the whole module is

    out[b,c,n] = sum_c' wp[c,c'] * q[b,c',n] * s[b,c'] + x[b,c,n]
    q = wq @ x_b          s[b,c'] = sum_n (wk@x_b)[c',n] * (wv@x_b)[c',n]

and the [b,h,N,N] attention tensor never needs to exist.  With
u = (wk+wv) @ x and d = (wk-wv) @ x:   s = (sum u^2 - sum d^2) / 4.

Sharding over 8 NeuronCores: core i handles batch b = i//4 and output
n-chunk j = i%4 (256 of the 1024 flattened h*w positions).  Each core
receives the full x_b (rotated so its own chunk comes first), computes
s_b redundantly, and writes its 64x256 output chunk.  No collectives:
an 8-core AllReduce has a ~10us latency floor.

v8 data path (fp16 on the wire, f32 accumulation in PSUM):
  - x ships fp16 in two [64,512] halves on the two HWDGE rings (SP +
    ACT) into one SBUF tensor.  Packed weights ride the same rings:
    [kv | wq^T wq^T] fp16 [64,256] goes FIRST on the ACT ring (so the
    uv matmuls are not gated by the slow-starting gpsimd ring), and
    the [wp^T; -wp^T] block [128,64] rides second on the SP ring (only
    needed late).  Four dynamic DMAs total; each pays ~1.5-2.5us fixed
    latency, so they all issue back-to-back at body start.
  - uv = Wkv @ x runs as two stacked fp16 matmuls; ACT squares each
    half straight out of PSUM with accum_out= (fused row-reduce).
    A dummy Square issued before any waits prefetches the PWP table.
  - q is computed REPLICATED onto 128 partitions (lhsT = [wq^T|wq^T]),
    and the final matmul contracts over 128 partitions against
    wpTs_ud = [wp^T; -wp^T] * redall * 0.25 -- this removes the
    cross-partition u/d subtract from the DVE chain entirely.
  - The residual "+x" is folded into the PSUM->SBUF eviction as DVE
    tensor_tensor adds, in two 128-column halves; each half's output
    DMA issues as soon as its half is evicted (lo on ACT, hi on SP).
  - There is NO nc.Block(): engine streams are emitted straight into
    the main body, so bass adds no end-of-body all-engine barrier and
    the compiler's teardown epilogue (which begins with its own
    all-engine sync) starts as soon as the last engine's body ends.
    The output DMAs' completion is not waited on anywhere: the
    multi-microsecond teardown (per-engine semaphore clears) covers
    the transfers, and nothing in the kernel consumes out_sem.
Measured end-to-end relative error ~8e-4 (threshold 2e-2).
"""
import numpy as np

import concourse.bass as bass
import concourse.mybir as mybir
from concourse.bass_utils import run_bass_kernel_spmd

F32 = mybir.dt.float32
F16 = mybir.dt.float16
MULT = mybir.AluOpType.mult
ADD = mybir.AluOpType.add
SQUARE = mybir.ActivationFunctionType.Square

B, C, H, W = 2, 64, 32, 32
N = H * W          # 1024
NCHUNK = N // 4    # 256 output columns per core


def _build_nc() -> bass.Bass:
    nc = bass.Bass()
    xlo_ext = nc.declare_dram_parameter("xlo", [64, 512], F16, isOutput=False)
    xhi_ext = nc.declare_dram_parameter("xhi", [64, 512], F16, isOutput=False)
    wa_ext = nc.declare_dram_parameter("wa", [64, 256], F16, isOutput=False)
    wb_ext = nc.declare_dram_parameter("wb", [128, 64], F16, isOutput=False)
    o_ext = nc.declare_dram_parameter("out", [64, 256], F16, isOutput=True)

    from contextlib import ExitStack

    with ExitStack() as ctx:
        e = ctx.enter_context
        X = e(nc.sbuf_tensor("X", [64, 1024], F16))
        Wa = e(nc.sbuf_tensor("Wa", [64, 256], F16))    # [kv | wqT wqT]
        Wb = e(nc.sbuf_tensor("Wb", [128, 64], F16))    # [wpT ; -wpT]
        sqb = e(nc.sbuf_tensor("sqb", [128, 1024], F16))  # square scratch
        redc = e(nc.sbuf_tensor("redc", [128, 2], F32))   # per-half row sums
        redall = e(nc.sbuf_tensor("redall", [128, 1], F32))
        Qsb = e(nc.sbuf_tensor("Qsb", [128, 256], F16))   # q replicated
        wpTs = e(nc.sbuf_tensor("wpTs", [128, 64], F16))
        Fsb = e(nc.sbuf_tensor("Fsb", [64, 256], F16))
        dummy = e(nc.sbuf_tensor("warmup", [1, 1], F32))
        uv1 = e(nc.psum_tensor("uv1", [128, 512], F32))
        uv2 = e(nc.psum_tensor("uv2", [128, 512], F32))
        Qp = e(nc.psum_tensor("Qp", [128, 256], F32))
        Op = e(nc.psum_tensor("Op", [64, 256], F32))
        xlo_sem = e(nc.semaphore("xlo_sem"))
        xhi_sem = e(nc.semaphore("xhi_sem"))
        wa_sem = e(nc.semaphore("wa_sem"))
        wb_sem = e(nc.semaphore("wb_sem"))
        pe_sem = e(nc.semaphore("pe_sem"))
        act_sem = e(nc.semaphore("act_sem"))
        dv_sem = e(nc.semaphore("dv_sem"))
        gp_sem = e(nc.semaphore("gp_sem"))
        out_sem = e(nc.semaphore("out_sem"))

        kv = Wa[:, 0:128]
        wq2 = Wa[:, 128:256]

        sync, gp, pe, act, dv = nc.sync, nc.gpsimd, nc.tensor, nc.scalar, nc.vector

        # ---- SP (sync): x low half + wp block in, first result half out ----
        sync.dma_start(X[:, 0:512], xlo_ext[:]).then_inc(xlo_sem, 16)
        sync.dma_start(Wb[:], wb_ext[:]).then_inc(wb_sem, 16)
        sync.wait_ge(dv_sem, 3)
        # completion covered by the NEFF teardown epilogue (see header)
        sync.dma_start(o_ext[:, 0:128], Fsb[:, 0:128]).then_inc(out_sem, 16)

        # ---- PE ----
        pe.wait_ge(wa_sem, 16)
        pe.wait_ge(xlo_sem, 16)
        # rows 0-63 = u = (wk+wv)x, rows 64-127 = d = (wk-wv)x
        pe.matmul(uv1[:], kv, X[:, 0:512], start=True, stop=True).then_inc(pe_sem, 1)
        pe.wait_ge(xhi_sem, 16)
        pe.matmul(uv2[:], kv, X[:, 512:1024], start=True, stop=True).then_inc(pe_sem, 1)
        # q replicated onto both partition halves
        pe.matmul(Qp[:], wq2, X[:, 0:256], start=True, stop=True).then_inc(pe_sem, 1)
        # out = (wp diag(s)) @ q via 128-deep contraction, two column halves
        pe.wait_ge(dv_sem, 2)
        pe.matmul(Op[:, 0:128], wpTs[:], Qsb[:, 0:128], start=True, stop=True).then_inc(pe_sem, 1)
        pe.matmul(Op[:, 128:256], wpTs[:], Qsb[:, 128:256], start=True, stop=True).then_inc(pe_sem, 1)

        # ---- ACT (scalar): weights + x high half in, fused squares, first out half ----
        act.dma_start(Wa[:], wa_ext[:]).then_inc(wa_sem, 16)
        act.dma_start(X[:, 512:1024], xhi_ext[:]).then_inc(xhi_sem, 16)
        # warm the ACT Square table while DMAs are in flight
        act.activation(dummy[:], nc.const_aps.tensor(0.0, (1, 1), F32), SQUARE).then_inc(act_sem, 1)
        act.wait_ge(pe_sem, 1)
        act.activation(sqb[:, 0:512], uv1[:], SQUARE, accum_out=redc[:, 0:1]).then_inc(act_sem, 1)
        act.wait_ge(pe_sem, 2)
        act.activation(sqb[:, 512:1024], uv2[:], SQUARE, accum_out=redc[:, 1:2]).then_inc(act_sem, 1)
        act.wait_ge(dv_sem, 4)
        act.dma_start(o_ext[:, 128:256], Fsb[:, 128:256]).then_inc(out_sem, 16)

        # ---- GpSimd: tiny redall = redc[:,0] + redc[:,1] in its idle stream ----
        gp.wait_ge(act_sem, 3)
        gp.tensor_tensor(redall[:], redc[:, 0:1], redc[:, 1:2], ADD).then_inc(gp_sem, 1)

        # ---- DVE (vector) ----
        # q PSUM -> SBUF fp16 in DVE's idle window
        dv.wait_ge(pe_sem, 3)
        dv.wait_ge(wb_sem, 16)
        dv.tensor_copy(Qsb[:], Qp[:]).then_inc(dv_sem, 1)
        dv.wait_ge(gp_sem, 1)
        # wpTs = [wpT; -wpT] * (su; sd) * 0.25  (the u/d subtract happens
        # inside the final matmul via the negated lower half)
        dv.tensor_scalar(wpTs[:], Wb[:], redall[:], 0.25, op0=MULT, op1=MULT).then_inc(dv_sem, 1)
        dv.wait_ge(pe_sem, 4)
        # evict out halves PSUM -> SBUF with the "+ x" residual folded in
        dv.tensor_tensor(Fsb[:, 0:128], Op[:, 0:128], X[:, 0:128], ADD).then_inc(dv_sem, 1)
        dv.wait_ge(pe_sem, 5)
        dv.tensor_tensor(Fsb[:, 128:256], Op[:, 128:256], X[:, 128:256], ADD).then_inc(dv_sem, 1)

    return nc


def _shard_inputs(x, wq, wk, wv, wp):
    """Full inputs -> list of 8 per-core {'xlo','xhi','wa','wb'} dicts (fp16)."""
    x = np.asarray(x, dtype=np.float32)
    wq, wk, wv, wp = (np.asarray(a, dtype=np.float32) for a in (wq, wk, wv, wp))
    xf = x.reshape(B, C, N)
    kv = np.concatenate([(wk + wv).T, (wk - wv).T], axis=1)                  # [64,128]
    wa = np.concatenate([kv, wq.T, wq.T], axis=1).astype(np.float16)         # [64,256]
    wa = np.ascontiguousarray(wa)
    wb = np.concatenate([wp.T, -wp.T], axis=0).astype(np.float16)            # [128,64]
    wb = np.ascontiguousarray(wb)
    in_maps = []
    for core in range(8):
        bb, j = core // 4, core % 4
        xr = np.roll(xf[bb], -j * NCHUNK, axis=1).astype(np.float16)         # [64,1024]
        in_maps.append({
            "xlo": np.ascontiguousarray(xr[:, 0:512]),
            "xhi": np.ascontiguousarray(xr[:, 512:1024]),
            "wa": wa,
            "wb": wb,
        })
    return in_maps


def _gather_outputs(results):
    """8 per-core {'out': [64,256] fp16} -> full [b,C,h,w] f32."""
    out = np.empty((B, C, N), dtype=np.float32)
    for core in range(8):
        bb, j = core // 4, core % 4
        out[bb, :, j * NCHUNK:(j + 1) * NCHUNK] = np.asarray(results[core]["out"]).astype(np.float32)
    return out.reshape(B, C, H, W)


_NC_CACHE = None


def kernel(x, wq, wk, wv, wp) -> np.ndarray:
    global _NC_CACHE
    if _NC_CACHE is None:
        _NC_CACHE = _build_nc()
    in_maps = _shard_inputs(x, wq, wk, wv, wp)
    last_err = None
    for _ in range(3):
        try:
            res = run_bass_kernel_spmd(_NC_CACHE, in_maps, core_ids=list(range(8)))
            return _gather_outputs(res.results)
        except Exception as exc:  # transient device-unrecoverable resets on retry
            last_err = exc
    raise last_err


# revision 22
# speedup vs baseline: 1.0736x; 1.0736x over previous
"""Distributed Trainium2 (Bass) kernel for nn_Attention_53764400611491.

The reference module has HEADS == C == 64, so head_dim d = C//HEADS = 1.
With d = 1 the attention algebra collapses: per (batch b, head c)

    attn = q k^T            (outer product, [N,N])
    o    = attn @ v  =  q * (k . v)        <- a scalar per (b,c)!

so the whole module is

    out[b,c,n] = sum_c' wp[c,c'] * q[b,c',n] * s[b,c'] + x[b,c,n]
    q = wq @ x_b          s[b,c'] = sum_n (wk@x_b)[c',n] * (wv@x_b)[c',n]

and the [b,h,N,N] attention tensor never needs to exist.  With
u = (wk+wv) @ x and d = (wk-wv) @ x:   s = (sum u^2 - sum d^2) / 4.

Sharding over 8 NeuronCores: core i handles batch b = i//4 and output
n-chunk j = i%4 (256 of the 1024 flattened h*w positions).  Each core
receives the full x_b (rotated so its own chunk comes first), computes
s_b redundantly, and writes its 64x256 output chunk.  No collectives:
an 8-core AllReduce has a ~10us latency floor.

v8 data path (fp16 on the wire, f32 accumulation in PSUM):
  - x ships fp16 in two [64,512] halves on the two HWDGE rings (SP +
    ACT) into one SBUF tensor.  Packed weights ride the same rings:
    [kv | wq^T wq^T] fp16 [64,256] goes FIRST on the ACT ring (so the
    uv matmuls are not gated by the slow-starting gpsimd ring), and
    the [wp^T; -wp^T] block [128,64] rides second on the SP ring (only
    needed late).  Four dynamic DMAs total; each pays ~1.5-2.5us fixed
    latency, so they all issue back-to-back at body start.
  - uv = Wkv @ x runs as two stacked fp16 matmuls; ACT squares each
    half straight out of PSUM with accum_out= (fused row-reduce).
    A dummy Square issued before any waits prefetches the PWP table.
  - q is computed REPLICATED onto 128 partitions (lhsT = [wq^T|wq^T]),
    and the final matmul contracts over 128 partitions against
    wpTs_ud = [wp^T; -wp^T] * redall * 0.25 -- this removes the
    cross-partition u/d subtract from the DVE chain entirely.
  - The residual "+x" is folded into the PSUM->SBUF eviction as DVE
    tensor_tensor adds, in two 128-column halves; each half's output
    DMA issues as soon as its half is evicted (lo on ACT, hi on SP).
  - There is NO nc.Block(): engine streams are emitted straight into
    the main body, so bass adds no end-of-body all-engine barrier and
    the compiler's teardown epilogue (which begins with its own
    all-engine sync) starts as soon as the last engine's body ends.
    The output DMAs' completion is not waited on anywhere: the
    multi-microsecond teardown (per-engine semaphore clears) covers
    the transfers, and nothing in the kernel consumes out_sem.
Measured end-to-end relative error ~8e-4 (threshold 2e-2).
"""
import numpy as np

import concourse.bass as bass
import concourse.mybir as mybir
from concourse.bass_utils import run_bass_kernel_spmd

F32 = mybir.dt.float32
F16 = mybir.dt.float16
MULT = mybir.AluOpType.mult
ADD = mybir.AluOpType.add
SQUARE = mybir.ActivationFunctionType.Square

B, C, H, W = 2, 64, 32, 32
N = H * W          # 1024
NCHUNK = N // 4    # 256 output columns per core


def _build_nc() -> bass.Bass:
    nc = bass.Bass()
    xlo_ext = nc.declare_dram_parameter("xlo", [64, 512], F16, isOutput=False)
    xhi_ext = nc.declare_dram_parameter("xhi", [64, 512], F16, isOutput=False)
    wa_ext = nc.declare_dram_parameter("wa", [64, 256], F16, isOutput=False)
    wb_ext = nc.declare_dram_parameter("wb", [128, 64], F16, isOutput=False)
    o_ext = nc.declare_dram_parameter("out", [64, 256], F16, isOutput=True)

    from contextlib import ExitStack

    with ExitStack() as ctx:
        e = ctx.enter_context
        X = e(nc.sbuf_tensor("X", [64, 1024], F16))
        Wa = e(nc.sbuf_tensor("Wa", [64, 256], F16))    # [kv | wqT wqT]
        Wb = e(nc.sbuf_tensor("Wb", [128, 64], F16))    # [wpT ; -wpT]
        sqb = e(nc.sbuf_tensor("sqb", [128, 1024], F16))  # square scratch
        redc = e(nc.sbuf_tensor("redc", [128, 2], F32))   # per-half row sums
        redall = e(nc.sbuf_tensor("redall", [128, 1], F32))
        Qsb = e(nc.sbuf_tensor("Qsb", [128, 256], F16))   # q replicated
        wpTs = e(nc.sbuf_tensor("wpTs", [128, 64], F16))
        Fsb = e(nc.sbuf_tensor("Fsb", [64, 256], F16))
        dummy = e(nc.sbuf_tensor("warmup", [1, 1], F32))
        uv1 = e(nc.psum_tensor("uv1", [128, 512], F32))
        uv2 = e(nc.psum_tensor("uv2", [128, 512], F32))
        Qp = e(nc.psum_tensor("Qp", [128, 256], F32))
        Op = e(nc.psum_tensor("Op", [64, 256], F32))
        xlo_sem = e(nc.semaphore("xlo_sem"))
        xhi_sem = e(nc.semaphore("xhi_sem"))
        wa_sem = e(nc.semaphore("wa_sem"))
        wb_sem = e(nc.semaphore("wb_sem"))
        pe_sem = e(nc.semaphore("pe_sem"))
        act_sem = e(nc.semaphore("act_sem"))
        dv_sem = e(nc.semaphore("dv_sem"))
        gp_sem = e(nc.semaphore("gp_sem"))
        out_sem = e(nc.semaphore("out_sem"))

        kv = Wa[:, 0:128]
        wq2 = Wa[:, 128:256]

        sync, gp, pe, act, dv = nc.sync, nc.gpsimd, nc.tensor, nc.scalar, nc.vector

        # ---- SP (sync): x low half + wp block in, first result half out ----
        sync.dma_start(X[:, 0:512], xlo_ext[:]).then_inc(xlo_sem, 16)
        sync.dma_start(Wb[:], wb_ext[:]).then_inc(wb_sem, 16)
        sync.wait_ge(dv_sem, 3)
        # completion covered by the NEFF teardown epilogue (see header)
        sync.dma_start(o_ext[:, 0:128], Fsb[:, 0:128]).then_inc(out_sem, 16)

        # ---- PE ----
        pe.wait_ge(wa_sem, 16)
        pe.wait_ge(xlo_sem, 16)
        # rows 0-63 = u = (wk+wv)x, rows 64-127 = d = (wk-wv)x
        pe.matmul(uv1[:], kv, X[:, 0:512], start=True, stop=True).then_inc(pe_sem, 1)
        pe.wait_ge(xhi_sem, 16)
        pe.matmul(uv2[:], kv, X[:, 512:1024], start=True, stop=True).then_inc(pe_sem, 1)
        # q replicated onto both partition halves
        pe.matmul(Qp[:], wq2, X[:, 0:256], start=True, stop=True).then_inc(pe_sem, 1)
        # out = (wp diag(s)) @ q via 128-deep contraction, two column halves
        pe.wait_ge(dv_sem, 2)
        pe.matmul(Op[:, 0:128], wpTs[:], Qsb[:, 0:128], start=True, stop=True).then_inc(pe_sem, 1)
        pe.matmul(Op[:, 128:256], wpTs[:], Qsb[:, 128:256], start=True, stop=True).then_inc(pe_sem, 1)

        # ---- ACT (scalar): weights + x high half in, fused squares, first out half ----
        act.dma_start(Wa[:], wa_ext[:]).then_inc(wa_sem, 16)
        act.dma_start(X[:, 512:1024], xhi_ext[:]).then_inc(xhi_sem, 16)
        # warm the ACT Square table while DMAs are in flight
        act.activation(dummy[:], nc.const_aps.tensor(0.0, (1, 1), F32), SQUARE).then_inc(act_sem, 1)
        act.wait_ge(pe_sem, 1)
        act.activation(sqb[:, 0:512], uv1[:], SQUARE, accum_out=redc[:, 0:1]).then_inc(act_sem, 1)
        act.wait_ge(pe_sem, 2)
        act.activation(sqb[:, 512:1024], uv2[:], SQUARE, accum_out=redc[:, 1:2]).then_inc(act_sem, 1)
        act.wait_ge(dv_sem, 4)
        act.dma_start(o_ext[:, 128:256], Fsb[:, 128:256]).then_inc(out_sem, 16)

        # ---- DVE (vector) ----
        # q PSUM -> SBUF fp16 in DVE's idle window
        dv.wait_ge(pe_sem, 3)
        dv.wait_ge(wb_sem, 16)
        dv.tensor_copy(Qsb[:], Qp[:]).then_inc(dv_sem, 1)
        dv.wait_ge(act_sem, 3)
        dv.reduce_sum(redall[:], redc[:], axis=mybir.AxisListType.X)
        dv.drain()  # redall landed (same-engine RAW)
        # wpTs = [wpT; -wpT] * (su; sd) * 0.25  (the u/d subtract happens
        # inside the final matmul via the negated lower half)
        dv.tensor_scalar(wpTs[:], Wb[:], redall[:], 0.25, op0=MULT, op1=MULT).then_inc(dv_sem, 1)
        dv.wait_ge(pe_sem, 4)
        # evict out halves PSUM -> SBUF with the "+ x" residual folded in
        dv.tensor_tensor(Fsb[:, 0:128], Op[:, 0:128], X[:, 0:128], ADD).then_inc(dv_sem, 1)
        dv.wait_ge(pe_sem, 5)
        dv.tensor_tensor(Fsb[:, 128:256], Op[:, 128:256], X[:, 128:256], ADD).then_inc(dv_sem, 1)

    return nc


def _shard_inputs(x, wq, wk, wv, wp):
    """Full inputs -> list of 8 per-core {'xlo','xhi','wa','wb'} dicts (fp16)."""
    x = np.asarray(x, dtype=np.float32)
    wq, wk, wv, wp = (np.asarray(a, dtype=np.float32) for a in (wq, wk, wv, wp))
    xf = x.reshape(B, C, N)
    kv = np.concatenate([(wk + wv).T, (wk - wv).T], axis=1)                  # [64,128]
    wa = np.concatenate([kv, wq.T, wq.T], axis=1).astype(np.float16)         # [64,256]
    wa = np.ascontiguousarray(wa)
    wb = np.concatenate([wp.T, -wp.T], axis=0).astype(np.float16)            # [128,64]
    wb = np.ascontiguousarray(wb)
    in_maps = []
    for core in range(8):
        bb, j = core // 4, core % 4
        xr = np.roll(xf[bb], -j * NCHUNK, axis=1).astype(np.float16)         # [64,1024]
        in_maps.append({
            "xlo": np.ascontiguousarray(xr[:, 0:512]),
            "xhi": np.ascontiguousarray(xr[:, 512:1024]),
            "wa": wa,
            "wb": wb,
        })
    return in_maps


def _gather_outputs(results):
    """8 per-core {'out': [64,256] fp16} -> full [b,C,h,w] f32."""
    out = np.empty((B, C, N), dtype=np.float32)
    for core in range(8):
        bb, j = core // 4, core % 4
        out[bb, :, j * NCHUNK:(j + 1) * NCHUNK] = np.asarray(results[core]["out"]).astype(np.float32)
    return out.reshape(B, C, H, W)


_NC_CACHE = None


def kernel(x, wq, wk, wv, wp) -> np.ndarray:
    global _NC_CACHE
    if _NC_CACHE is None:
        _NC_CACHE = _build_nc()
    in_maps = _shard_inputs(x, wq, wk, wv, wp)
    last_err = None
    for _ in range(3):
        try:
            res = run_bass_kernel_spmd(_NC_CACHE, in_maps, core_ids=list(range(8)))
            return _gather_outputs(res.results)
        except Exception as exc:  # transient device-unrecoverable resets on retry
            last_err = exc
    raise last_err


# revision 44
# speedup vs baseline: 1.0988x; 1.0234x over previous
"""Distributed Trainium2 (Bass) kernel for nn_Attention_53764400611491.

The reference module has HEADS == C == 64, so head_dim d = C//HEADS = 1.
With d = 1 the attention algebra collapses: per (batch b, head c)

    attn = q k^T            (outer product, [N,N])
    o    = attn @ v  =  q * (k . v)        <- a scalar per (b,c)!

so the whole module is

    out[b,c,n] = sum_c' wp[c,c'] * q[b,c',n] * s[b,c'] + x[b,c,n]
    q = wq @ x_b          s[b,c'] = sum_n (wk@x_b)[c',n] * (wv@x_b)[c',n]

and the [b,h,N,N] attention tensor never needs to exist.  With
u = (wk+wv) @ x and d = (wk-wv) @ x:   s = (sum u^2 - sum d^2) / 4.

Sharding over 8 NeuronCores: core i handles batch b = i//4 and output
n-chunk j = i%4 (256 of the 1024 flattened h*w positions).  Each core
receives the full x_b (rotated so its own chunk comes first), computes
s_b redundantly, and writes its 64x256 output chunk.  No collectives:
an 8-core AllReduce has a ~10us latency floor.

Data path (fp16 on the wire, f32 accumulation in PSUM):
  - x ships fp16 in two [64,512] halves on the two HWDGE rings (SP +
    ACT) into one SBUF tensor.  Packed weights ride the ACT ring:
    [kv | wq^T wq^T] fp16 [64,256] goes FIRST (so the uv matmuls are
    not gated by the slow-starting gpsimd ring) and the small
    0.25*[wp^T; -wp^T] block [128,64] rides third (only needed ~2us
    later).  The critical xlo half has the SP ring to itself.  xlo and
    the main weights share ONE semaphore so the PE retires a single
    wait.  Four dynamic DMAs total; each pays ~1.5-2.5us fixed
    latency, so they all issue back-to-back at body start.
  - uv = Wkv @ x runs as two stacked fp16 matmuls; ACT squares each
    half straight out of PSUM with accum_out= (fused row-reduce).
    A dummy Square issued before any waits prefetches the PWP table.
  - q is computed REPLICATED onto 128 partitions (lhsT = [wq^T|wq^T]),
    and the final matmul contracts over 128 partitions against
    wpTs = Wb*redc0 + Wb*redc1 (Wb = 0.25*[wp^T;-wp^T]) -- this
    removes the cross-partition u/d subtract entirely.  The first
    partial product is computed right after the first square, off the
    critical path; after the last square a single fused
    scalar_tensor_tensor finishes wpTs (no reduce, no extra drain).
  - The residual "+x" is folded into the PSUM->SBUF eviction as DVE
    tensor_tensor adds, in two 128-column halves; each half's output
    DMA issues as soon as its half is evicted (lo on SP, hi on ACT).
  - There is NO nc.Block(): engine streams are emitted straight into
    the main body, so bass adds no end-of-body all-engine barrier and
    the compiler's teardown epilogue (which begins with its own
    all-engine sync) starts as soon as the last engine's body ends.
    The output DMAs' completion is not waited on anywhere: the
    multi-microsecond teardown (per-engine semaphore clears) covers
    the transfers, and nothing in the kernel consumes out_sem.
Measured end-to-end relative error ~8e-4 (threshold 2e-2).
"""
import numpy as np

import concourse.bass as bass
import concourse.mybir as mybir
from concourse.bass_utils import run_bass_kernel_spmd

F32 = mybir.dt.float32
F16 = mybir.dt.float16
MULT = mybir.AluOpType.mult
ADD = mybir.AluOpType.add
SQUARE = mybir.ActivationFunctionType.Square

B, C, H, W = 2, 64, 32, 32
N = H * W          # 1024
NCHUNK = N // 4    # 256 output columns per core


def _build_nc() -> bass.Bass:
    nc = bass.Bass()
    xlo_ext = nc.declare_dram_parameter("xlo", [64, 512], F16, isOutput=False)
    xhi_ext = nc.declare_dram_parameter("xhi", [64, 512], F16, isOutput=False)
    wa_ext = nc.declare_dram_parameter("wa", [64, 256], F16, isOutput=False)
    wb_ext = nc.declare_dram_parameter("wb", [128, 64], F16, isOutput=False)
    o_ext = nc.declare_dram_parameter("out", [64, 256], F16, isOutput=True)

    from contextlib import ExitStack

    with ExitStack() as ctx:
        e = ctx.enter_context
        X = e(nc.sbuf_tensor("X", [64, 1024], F16))
        Wa = e(nc.sbuf_tensor("Wa", [64, 256], F16))    # [kv | wqT wqT]
        Wb = e(nc.sbuf_tensor("Wb", [128, 64], F16))    # [wpT ; -wpT]
        sqb = e(nc.sbuf_tensor("sqb", [128, 1024], F16))  # square scratch
        redc = e(nc.sbuf_tensor("redc", [128, 2], F32))   # per-half row sums
        Qsb = e(nc.sbuf_tensor("Qsb", [128, 256], F16))   # q replicated
        wpTsA = e(nc.sbuf_tensor("wpTsA", [128, 64], F16))
        wpTs = e(nc.sbuf_tensor("wpTs", [128, 64], F16))
        Fsb = e(nc.sbuf_tensor("Fsb", [64, 256], F16))
        dummy = e(nc.sbuf_tensor("warmup", [1, 1], F32))
        uv1 = e(nc.psum_tensor("uv1", [128, 512], F32))
        uv2 = e(nc.psum_tensor("uv2", [128, 512], F32))
        Qp = e(nc.psum_tensor("Qp", [128, 256], F32))
        Op = e(nc.psum_tensor("Op", [64, 256], F32))
        xw_sem = e(nc.semaphore("xw_sem"))
        xhi_sem = e(nc.semaphore("xhi_sem"))
        wb_sem = e(nc.semaphore("wb_sem"))
        pe_sem = e(nc.semaphore("pe_sem"))
        act_sem = e(nc.semaphore("act_sem"))
        dv_sem = e(nc.semaphore("dv_sem"))
        out_sem = e(nc.semaphore("out_sem"))

        kv = Wa[:, 0:128]
        wq2 = Wa[:, 128:256]

        sync, gp, pe, act, dv = nc.sync, nc.gpsimd, nc.tensor, nc.scalar, nc.vector

        # ---- SP (sync): x low half in, first result half out ----
        # xlo and wa share one semaphore so the PE retires a single wait
        sync.dma_start(X[:, 0:512], xlo_ext[:]).then_inc(xw_sem, 16)
        # wp block rides second here: it is needed ~2us later than x, and the
        # ACT ring must stay 2-deep so its PWP table load + warmup finish
        # before the first square (ACT's stream is serial).
        sync.dma_start(Wb[:], wb_ext[:]).then_inc(wb_sem, 16)
        # Sync takes the LATER output half: it sits late (position 4) in the
        # teardown barrier's fixed increment ripple (Scalar first), so its
        # late arrival hides; ACT's would stall the whole ripple.
        sync.wait_ge(dv_sem, 4)
        # completion covered by the NEFF teardown epilogue (see header)
        sync.dma_start(o_ext[:, 128:256], Fsb[:, 128:256]).then_inc(out_sem, 16)

        # ---- PE ----
        pe.wait_ge(xw_sem, 32)
        # rows 0-63 = u = (wk+wv)x, rows 64-127 = d = (wk-wv)x
        pe.matmul(uv1[:], kv, X[:, 0:512], start=True, stop=True).then_inc(pe_sem, 1)
        pe.wait_ge(xhi_sem, 16)
        pe.matmul(uv2[:], kv, X[:, 512:1024], start=True, stop=True).then_inc(pe_sem, 1)
        # q replicated onto both partition halves
        pe.matmul(Qp[:], wq2, X[:, 0:256], start=True, stop=True).then_inc(pe_sem, 1)
        # out = (wp diag(s)) @ q via 128-deep contraction, two column halves
        pe.wait_ge(dv_sem, 2)
        pe.matmul(Op[:, 0:128], wpTs[:], Qsb[:, 0:128], start=True, stop=True).then_inc(pe_sem, 1)
        pe.matmul(Op[:, 128:256], wpTs[:], Qsb[:, 128:256], start=True, stop=True).then_inc(pe_sem, 1)

        # ---- ACT (scalar): weights + x high half in, fused squares, second out half ----
        act.dma_start(Wa[:], wa_ext[:]).then_inc(xw_sem, 16)
        act.dma_start(X[:, 512:1024], xhi_ext[:]).then_inc(xhi_sem, 16)
        # warm the ACT Square table while DMAs are in flight
        act.activation(dummy[:], nc.const_aps.tensor(0.0, (1, 1), F32), SQUARE).then_inc(act_sem, 1)
        act.wait_ge(pe_sem, 1)
        act.activation(sqb[:, 0:512], uv1[:], SQUARE, accum_out=redc[:, 0:1]).then_inc(act_sem, 1)
        act.wait_ge(pe_sem, 2)
        act.activation(sqb[:, 512:1024], uv2[:], SQUARE, accum_out=redc[:, 1:2]).then_inc(act_sem, 1)
        act.wait_ge(dv_sem, 3)
        act.dma_start(o_ext[:, 0:128], Fsb[:, 0:128]).then_inc(out_sem, 16)

        # ---- DVE (vector) ----
        # q PSUM -> SBUF fp16 in DVE's idle window
        dv.wait_ge(pe_sem, 3)
        dv.wait_ge(wb_sem, 16)
        dv.tensor_copy(Qsb[:], Qp[:]).then_inc(dv_sem, 1)
        # first-half partial wpTsA = Wb*redc0, off the critical path (Wb
        # carries the 0.25/4 factor from the +- identity, folded host-side)
        dv.wait_ge(act_sem, 2)
        dv.tensor_scalar(wpTsA[:], Wb[:], redc[:, 0:1], None, op0=MULT)
        dv.drain()  # wpTsA landed (same-engine RAW); hidden under sq_hi
        # wpTs = Wb*redc1 + wpTsA in one fused op right after the last square
        dv.wait_ge(act_sem, 3)
        dv.scalar_tensor_tensor(wpTs[:], Wb[:], redc[:, 1:2], wpTsA[:],
                                MULT, ADD).then_inc(dv_sem, 1)
        dv.wait_ge(pe_sem, 4)
        # evict out halves PSUM -> SBUF with the "+ x" residual folded in
        dv.tensor_tensor(Fsb[:, 0:128], Op[:, 0:128], X[:, 0:128], ADD).then_inc(dv_sem, 1)
        dv.wait_ge(pe_sem, 5)
        dv.tensor_tensor(Fsb[:, 128:256], Op[:, 128:256], X[:, 128:256], ADD).then_inc(dv_sem, 1)

    return nc


def _shard_inputs(x, wq, wk, wv, wp):
    """Full inputs -> list of 8 per-core {'xlo','xhi','wa','wb'} dicts (fp16)."""
    x = np.asarray(x, dtype=np.float32)
    wq, wk, wv, wp = (np.asarray(a, dtype=np.float32) for a in (wq, wk, wv, wp))
    xf = x.reshape(B, C, N)
    kv = np.concatenate([(wk + wv).T, (wk - wv).T], axis=1)                  # [64,128]
    wa = np.concatenate([kv, wq.T, wq.T], axis=1).astype(np.float16)         # [64,256]
    wa = np.ascontiguousarray(wa)
    wb = (0.25 * np.concatenate([wp.T, -wp.T], axis=0)).astype(np.float16)   # [128,64]
    wb = np.ascontiguousarray(wb)
    in_maps = []
    for core in range(8):
        bb, j = core // 4, core % 4
        xr = np.roll(xf[bb], -j * NCHUNK, axis=1).astype(np.float16)         # [64,1024]
        in_maps.append({
            "xlo": np.ascontiguousarray(xr[:, 0:512]),
            "xhi": np.ascontiguousarray(xr[:, 512:1024]),
            "wa": wa,
            "wb": wb,
        })
    return in_maps


def _gather_outputs(results):
    """8 per-core {'out': [64,256] fp16} -> full [b,C,h,w] f32."""
    out = np.empty((B, C, N), dtype=np.float32)
    for core in range(8):
        bb, j = core // 4, core % 4
        out[bb, :, j * NCHUNK:(j + 1) * NCHUNK] = np.asarray(results[core]["out"]).astype(np.float32)
    return out.reshape(B, C, H, W)


_NC_CACHE = None


def kernel(x, wq, wk, wv, wp) -> np.ndarray:
    global _NC_CACHE
    if _NC_CACHE is None:
        _NC_CACHE = _build_nc()
    in_maps = _shard_inputs(x, wq, wk, wv, wp)
    last_err = None
    # Transient NRT_EXEC_UNIT_UNRECOVERABLE wedges have been observed to
    # persist across immediate in-process retries but clear after a short
    # wait (terminal-side recovery), so back off between attempts.
    import time
    for delay in (0.0, 1.0, 3.0, 8.0):
        time.sleep(delay)
        try:
            res = run_bass_kernel_spmd(_NC_CACHE, in_maps, core_ids=list(range(8)))
            return _gather_outputs(res.results)
        except Exception as exc:
            last_err = exc
    raise last_err


# revision 46
# speedup vs baseline: 1.3326x; 1.2128x over previous
"""Distributed Trainium2 (Bass) kernel for nn_Attention_53764400611491.

The reference module has HEADS == C == 64, so head_dim d = C//HEADS = 1.
With d = 1 the attention algebra collapses: per (batch b, head c)

    attn = q k^T            (outer product, [N,N])
    o    = attn @ v  =  q * (k . v)        <- a scalar per (b,c)!

so the whole module is

    out[b,c,n] = sum_c' wp[c,c'] * q[b,c',n] * s[b,c'] + x[b,c,n]
    q = wq @ x_b          s[b,c'] = sum_n (wk@x_b)[c',n] * (wv@x_b)[c',n]

and the [b,h,N,N] attention tensor never needs to exist.  With
u = (wk+wv) @ x and d = (wk-wv) @ x:   s = (sum u^2 - sum d^2) / 4.

Sharding over 8 NeuronCores: core i handles batch b = i//4 and output
n-chunk j = i%4 (256 of the 1024 flattened h*w positions).  Each core
receives the full x_b (rotated so its own chunk comes first), computes
s_b redundantly, and writes its 64x256 output chunk.  No collectives:
an 8-core AllReduce has a ~10us latency floor.

Data path (fp16 on the wire, f32 accumulation in PSUM):
  - x ships fp16 in two [64,512] halves on the two HWDGE rings (SP +
    ACT) into one SBUF tensor.  Packed weights ride the ACT ring:
    [kv | wq^T wq^T] fp16 [64,256] goes FIRST (so the uv matmuls are
    not gated by the slow-starting gpsimd ring) and the small
    0.25*[wp^T; -wp^T] block [128,64] rides third (only needed ~2us
    later).  The critical xlo half has the SP ring to itself.  xlo and
    the main weights share ONE semaphore so the PE retires a single
    wait.  Four dynamic DMAs total; each pays ~1.5-2.5us fixed
    latency, so they all issue back-to-back at body start.
  - uv = Wkv @ x runs as two stacked fp16 matmuls; ACT squares each
    half straight out of PSUM with accum_out= (fused row-reduce).
    A dummy Square issued before any waits prefetches the PWP table.
  - q is computed REPLICATED onto 128 partitions (lhsT = [wq^T|wq^T]),
    and the final matmul contracts over 128 partitions against
    wpTs = Wb*redc0 + Wb*redc1 (Wb = 0.25*[wp^T;-wp^T]) -- this
    removes the cross-partition u/d subtract entirely.  The first
    partial product is computed right after the first square, off the
    critical path; after the last square a single fused
    scalar_tensor_tensor finishes wpTs (no reduce, no extra drain).
  - The residual "+x" is folded into the PSUM->SBUF eviction as DVE
    tensor_tensor adds, in two 128-column halves; each half's output
    DMA issues as soon as its half is evicted (lo on SP, hi on ACT).
  - There is NO nc.Block(): engine streams are emitted straight into
    the main body, so bass adds no end-of-body all-engine barrier and
    the compiler's teardown epilogue (which begins with its own
    all-engine sync) starts as soon as the last engine's body ends.
    The output DMAs' completion is not waited on anywhere: the
    multi-microsecond teardown (per-engine semaphore clears) covers
    the transfers, and nothing in the kernel consumes out_sem.
Measured end-to-end relative error ~8e-4 (threshold 2e-2).
"""
import numpy as np

import concourse.bass as bass
import concourse.mybir as mybir
from concourse.bass_utils import run_bass_kernel_spmd

F32 = mybir.dt.float32
F16 = mybir.dt.float16
MULT = mybir.AluOpType.mult
ADD = mybir.AluOpType.add
SQUARE = mybir.ActivationFunctionType.Square

B, C, H, W = 2, 64, 32, 32
N = H * W          # 1024
NCHUNK = N // 4    # 256 output columns per core


def _build_nc() -> bass.Bass:
    nc = bass.Bass()
    xlo_ext = nc.declare_dram_parameter("xlo", [64, 512], F16, isOutput=False)
    xhi_ext = nc.declare_dram_parameter("xhi", [64, 512], F16, isOutput=False)
    wa_ext = nc.declare_dram_parameter("wa", [64, 256], F16, isOutput=False)
    wb_ext = nc.declare_dram_parameter("wb", [128, 64], F16, isOutput=False)
    o_ext = nc.declare_dram_parameter("out", [64, 256], F16, isOutput=True)

    from contextlib import ExitStack

    with ExitStack() as ctx:
        e = ctx.enter_context
        X = e(nc.sbuf_tensor("X", [64, 1024], F16))
        Wa = e(nc.sbuf_tensor("Wa", [64, 256], F16))    # [kv | wqT wqT]
        Wb = e(nc.sbuf_tensor("Wb", [128, 64], F16))    # [wpT ; -wpT]
        sqb = e(nc.sbuf_tensor("sqb", [128, 1024], F16))  # square scratch
        redc = e(nc.sbuf_tensor("redc", [128, 2], F32))   # per-half row sums
        Qsb = e(nc.sbuf_tensor("Qsb", [128, 256], F16))   # q replicated
        wpTsA = e(nc.sbuf_tensor("wpTsA", [128, 64], F16))
        wpTs = e(nc.sbuf_tensor("wpTs", [128, 64], F16))
        Fsb = e(nc.sbuf_tensor("Fsb", [64, 256], F16))
        dummy = e(nc.sbuf_tensor("warmup", [1, 1], F32))
        uv1 = e(nc.psum_tensor("uv1", [128, 512], F32))
        uv2 = e(nc.psum_tensor("uv2", [128, 512], F32))
        Qp = e(nc.psum_tensor("Qp", [128, 256], F32))
        Op = e(nc.psum_tensor("Op", [64, 256], F32))
        xw_sem = e(nc.semaphore("xw_sem"))
        xhi_sem = e(nc.semaphore("xhi_sem"))
        wb_sem = e(nc.semaphore("wb_sem"))
        pe_sem = e(nc.semaphore("pe_sem"))
        act_sem = e(nc.semaphore("act_sem"))
        dv_sem = e(nc.semaphore("dv_sem"))
        out_sem = e(nc.semaphore("out_sem"))

        kv = Wa[:, 0:128]
        wq2 = Wa[:, 128:256]

        sync, gp, pe, act, dv = nc.sync, nc.gpsimd, nc.tensor, nc.scalar, nc.vector

        # ---- SP (sync): x both halves + wp block in, second result half out ----
        # xlo and wa (on the ACT ring) share one semaphore so the PE retires
        # a single wait.  xhi rides second here rather than on ACT: it has
        # ~0.7us of PE-serial slack, and keeping the ACT ring to ONE issue
        # lets ACT's serial PWP-table-load + warmup finish ~0.9us before the
        # first square instead of racing it by ~50ns.
        sync.dma_start(X[:, 0:512], xlo_ext[:]).then_inc(xw_sem, 16)
        sync.dma_start(X[:, 512:1024], xhi_ext[:]).then_inc(xhi_sem, 16)
        # wp block is needed ~2us later still
        sync.dma_start(Wb[:], wb_ext[:]).then_inc(wb_sem, 16)
        # Sync takes the LATER output half: it sits late (position 4) in the
        # teardown barrier's fixed increment ripple (Scalar first), so its
        # late arrival hides; ACT's would stall the whole ripple.
        sync.wait_ge(dv_sem, 4)
        # completion covered by the NEFF teardown epilogue (see header)
        sync.dma_start(o_ext[:, 128:256], Fsb[:, 128:256]).then_inc(out_sem, 16)

        # ---- PE ----
        pe.wait_ge(xw_sem, 32)
        # rows 0-63 = u = (wk+wv)x, rows 64-127 = d = (wk-wv)x
        pe.matmul(uv1[:], kv, X[:, 0:512], start=True, stop=True).then_inc(pe_sem, 1)
        pe.wait_ge(xhi_sem, 16)
        pe.matmul(uv2[:], kv, X[:, 512:1024], start=True, stop=True).then_inc(pe_sem, 1)
        # q replicated onto both partition halves
        pe.matmul(Qp[:], wq2, X[:, 0:256], start=True, stop=True).then_inc(pe_sem, 1)
        # out = (wp diag(s)) @ q via 128-deep contraction, two column halves
        pe.wait_ge(dv_sem, 2)
        pe.matmul(Op[:, 0:128], wpTs[:], Qsb[:, 0:128], start=True, stop=True).then_inc(pe_sem, 1)
        pe.matmul(Op[:, 128:256], wpTs[:], Qsb[:, 128:256], start=True, stop=True).then_inc(pe_sem, 1)

        # ---- ACT (scalar): weights in, fused squares, first out half ----
        act.dma_start(Wa[:], wa_ext[:]).then_inc(xw_sem, 16)
        # warm the ACT Square table while DMAs are in flight
        act.activation(dummy[:], nc.const_aps.tensor(0.0, (1, 1), F32), SQUARE).then_inc(act_sem, 1)
        act.wait_ge(pe_sem, 1)
        act.activation(sqb[:, 0:512], uv1[:], SQUARE, accum_out=redc[:, 0:1]).then_inc(act_sem, 1)
        act.wait_ge(pe_sem, 2)
        act.activation(sqb[:, 512:1024], uv2[:], SQUARE, accum_out=redc[:, 1:2]).then_inc(act_sem, 1)
        act.wait_ge(dv_sem, 3)
        act.dma_start(o_ext[:, 0:128], Fsb[:, 0:128]).then_inc(out_sem, 16)

        # ---- DVE (vector) ----
        # q PSUM -> SBUF fp16 in DVE's idle window
        dv.wait_ge(pe_sem, 3)
        dv.wait_ge(wb_sem, 16)
        dv.tensor_copy(Qsb[:], Qp[:]).then_inc(dv_sem, 1)
        # first-half partial wpTsA = Wb*redc0, off the critical path (Wb
        # carries the 0.25/4 factor from the +- identity, folded host-side)
        dv.wait_ge(act_sem, 2)
        dv.tensor_scalar(wpTsA[:], Wb[:], redc[:, 0:1], None, op0=MULT)
        dv.drain()  # wpTsA landed (same-engine RAW); hidden under sq_hi
        # wpTs = Wb*redc1 + wpTsA in one fused op right after the last square
        dv.wait_ge(act_sem, 3)
        dv.scalar_tensor_tensor(wpTs[:], Wb[:], redc[:, 1:2], wpTsA[:],
                                MULT, ADD).then_inc(dv_sem, 1)
        dv.wait_ge(pe_sem, 4)
        # evict out halves PSUM -> SBUF with the "+ x" residual folded in
        dv.tensor_tensor(Fsb[:, 0:128], Op[:, 0:128], X[:, 0:128], ADD).then_inc(dv_sem, 1)
        dv.wait_ge(pe_sem, 5)
        dv.tensor_tensor(Fsb[:, 128:256], Op[:, 128:256], X[:, 128:256], ADD).then_inc(dv_sem, 1)

    return nc


def _shard_inputs(x, wq, wk, wv, wp):
    """Full inputs -> list of 8 per-core {'xlo','xhi','wa','wb'} dicts (fp16)."""
    x = np.asarray(x, dtype=np.float32)
    wq, wk, wv, wp = (np.asarray(a, dtype=np.float32) for a in (wq, wk, wv, wp))
    xf = x.reshape(B, C, N)
    kv = np.concatenate([(wk + wv).T, (wk - wv).T], axis=1)                  # [64,128]
    wa = np.concatenate([kv, wq.T, wq.T], axis=1).astype(np.float16)         # [64,256]
    wa = np.ascontiguousarray(wa)
    wb = (0.25 * np.concatenate([wp.T, -wp.T], axis=0)).astype(np.float16)   # [128,64]
    wb = np.ascontiguousarray(wb)
    in_maps = []
    for core in range(8):
        bb, j = core // 4, core % 4
        xr = np.roll(xf[bb], -j * NCHUNK, axis=1).astype(np.float16)         # [64,1024]
        in_maps.append({
            "xlo": np.ascontiguousarray(xr[:, 0:512]),
            "xhi": np.ascontiguousarray(xr[:, 512:1024]),
            "wa": wa,
            "wb": wb,
        })
    return in_maps


def _gather_outputs(results):
    """8 per-core {'out': [64,256] fp16} -> full [b,C,h,w] f32."""
    out = np.empty((B, C, N), dtype=np.float32)
    for core in range(8):
        bb, j = core // 4, core % 4
        out[bb, :, j * NCHUNK:(j + 1) * NCHUNK] = np.asarray(results[core]["out"]).astype(np.float32)
    return out.reshape(B, C, H, W)


_NC_CACHE = None


def kernel(x, wq, wk, wv, wp) -> np.ndarray:
    global _NC_CACHE
    if _NC_CACHE is None:
        _NC_CACHE = _build_nc()
    in_maps = _shard_inputs(x, wq, wk, wv, wp)
    last_err = None
    # Transient NRT_EXEC_UNIT_UNRECOVERABLE wedges have been observed to
    # persist across immediate in-process retries but clear after a short
    # wait (terminal-side recovery), so back off between attempts.
    import time
    for delay in (0.0, 1.0, 3.0, 8.0):
        time.sleep(delay)
        try:
            res = run_bass_kernel_spmd(_NC_CACHE, in_maps, core_ids=list(range(8)))
            return _gather_outputs(res.results)
        except Exception as exc:
            last_err = exc
    raise last_err
